# revision 25
# baseline (speedup 1.0000x reference)
"""Causal self-attention (B=2, T=2048, C=1024, H=16) on 8 TRN2 NeuronCores.

Megatron-style tensor parallelism over heads: each core computes 2 of the 16
heads (Wq/Wk/Wv column-sharded, Wo row-sharded) and produces a partial output
projection; the partials are summed on the host (the all-reduce).

v2d: software-pipelined emission.  The attention inner loop (S matmul -> exp
on ACT -> PV matmul) is pipelined one j-tile deep so the PE rarely waits for
the scalar engine.  Projection work (hard deadline: next block) and
output-projection work (soft deadline: end of kernel) are woven into the
attention slots from two queues to fill the PE's ACT-bound slack and keep the
HAM clock-gate warm; soft units are gated to slot>=2 so they never head-of-
line block the PE behind the previous block's epilogue.  Diagonal j-tiles are
trimmed to their causal extent (S/PV moving dim 512-128q; exp runs full-tile,
stale columns are never read) with a 128x128 triangle mask-multiply on DVE.
V tiles are built with DMA XBAR transposes (no PE transpose / DVE copies).
The softmax 1/s broadcast is a rank-1 fp32r PE matmul.  xT tile DMAs are
issued two blocks ahead.

Per-core device dataflow (everything kept transposed so the PE contraction dim
is always the partition dim):
  xT [C, B*T] (host-pretransposed, replicated)
  QT/KT/VT = W_locT.T @ xT  (bf16 matmuls, K-tiled over C)
  V tiles   = DMA-transpose of VT, with a ones-column appended per head
  S^T       = K_loc @ Q_loc^T per (batch, head, 128-j-tile, 512-i-block),
              2 heads in concurrent PE row-groups (contraction = D = 64)
  P^T       = exp(S^T / 8) on ACT (no max-subtraction needed: |S| small)
  O^T|s     = [V|1].T @ P^T accumulated over j (ones row gives softmax sums)
  ylocT     = O^T * (1/s)  (reciprocal + rank-1 PE broadcast of 1/s)
  yT_part   = Wo_locT.T @ ylocT   -> DRAM [C, B*T]
Host: y = (sum_cores yT_part).T + bo, reshape to [B, T, C].
"""

import sys
from collections import deque

if "/opt/trn_rl_repo" not in sys.path:
    sys.path.insert(0, "/opt/trn_rl_repo")

import numpy as np

import concourse.bass as bass
import concourse.tile as tile
from concourse import bacc
from concourse import mybir
from concourse.bass_utils import run_bass_kernel_spmd

F32 = mybir.dt.float32
F32R = mybir.dt.float32r
BF16 = mybir.dt.bfloat16
U32 = mybir.dt.uint32
AF = mybir.ActivationFunctionType
ALU = mybir.AluOpType

B, T, C, H = 2, 2048, 1024, 16
D = C // H          # 64
NCORES = 8
HL = H // NCORES    # 2 local heads
CL = C // NCORES    # 128 local channels
BT = B * T          # 4096
TB = 512            # t-block (matmul moving width)
NTB = BT // TB      # 8
NKT = C // 128      # 8 contraction tiles for projections
IB = T // TB        # 4 i-blocks per batch
NJT = T // 128      # 16 j-tiles per batch
VW = 130            # V tile width: 2 heads x (64 V cols + 1 ones col)
TOTAL_SLOTS = 2 * (4 + 8 + 12 + 16)   # 80 attention j-tile slots


def build_nc() -> bass.Bass:
    nc = bacc.Bacc()

    xT_d = nc.declare_dram_parameter("xT", [C, BT], BF16, isOutput=False)
    wqT_d = nc.declare_dram_parameter("wqT", [128, C], BF16, isOutput=False)
    wkT_d = nc.declare_dram_parameter("wkT", [128, C], BF16, isOutput=False)
    wvT_d = nc.declare_dram_parameter("wvT", [128, C], BF16, isOutput=False)
    woT_d = nc.declare_dram_parameter("woT", [CL, C], BF16, isOutput=False)
    bq_d = nc.declare_dram_parameter("bq", [CL, 1], F32, isOutput=False)
    bk_d = nc.declare_dram_parameter("bk", [CL, 1], F32, isOutput=False)
    bv_d = nc.declare_dram_parameter("bv", [CL, 1], F32, isOutput=False)
    tri_d = nc.declare_dram_parameter("tri", [128, 128], BF16, isOutput=False)
    id_d = nc.declare_dram_parameter("ident", [128, 128], BF16, isOutput=False)
    yT_d = nc.declare_dram_parameter("yT", [C, BT], BF16, isOutput=True)

    with tile.TileContext(nc) as tc:
        with (
            tc.tile_pool(name="const", bufs=1) as const,
            tc.tile_pool(name="work", bufs=2) as work,
            tc.tile_pool(name="psum", bufs=2, space="PSUM") as psum,
        ):
            # ---------------- persistent SBUF state ------------------------
            wq_sb = const.tile([128, C], BF16)
            wk_sb = const.tile([128, C], BF16)
            wv_sb = const.tile([128, C], BF16)
            wo_sb = const.tile([128, C], BF16)
            tri_sb = const.tile([128, 128], BF16)
            bq_sb = const.tile([128, 1], F32)
            bk_sb = const.tile([128, 1], F32)
            bv_sb = const.tile([128, 1], F32)
            id_sb = const.tile([128, 128], BF16)
            QT = const.tile([128, BT], BF16)
            KT = const.tile([128, BT], BF16)
            ylocT = const.tile([128, BT], BF16)
            V = const.tile([128, (BT // 128) * VW], BF16)

            xts = {}

            def emit_xt_dma(tb, chunks=(0, 1), qn=2):
                # qn splits the tile into qn equal ct-range chunks
                tcols = slice(tb * TB, (tb + 1) * TB)
                if tb not in xts:
                    xts[tb] = work.tile([128, NKT * TB], BF16, tag="xt",
                                        bufs=4, name=f"xt_{tb}")
                xv = xT_d[:, tcols].rearrange("(ct p) t -> p ct t", p=128)
                xo = xts[tb][:, :].rearrange("p (ct t) -> p ct t", ct=NKT)
                hn = NKT // qn
                for c in chunks:
                    nc.sync.dma_start(xo[:, c * hn:(c + 1) * hn, :],
                                      xv[:, c * hn:(c + 1) * hn, :])

            # initial loads, ordered so tb0's projection can start early
            nc.sync.dma_start(wq_sb[:, :], wqT_d[:, :])
            emit_xt_dma(0, chunks=(0, 1), qn=4)
            nc.sync.dma_start(wk_sb[:, :], wkT_d[:, :])
            nc.sync.dma_start(wv_sb[:, :], wvT_d[:, :])
            emit_xt_dma(0, chunks=(2, 3), qn=4)
            nc.sync.dma_start(bq_sb[:, :], bq_d[:, :])
            nc.sync.dma_start(bk_sb[:, :], bk_d[:, :])
            nc.sync.dma_start(bv_sb[:, :], bv_d[:, :])
            nc.sync.dma_start(tri_sb[:, :], tri_d[:, :])
            nc.sync.dma_start(wo_sb[:, :], woT_d[:, :])
            nc.sync.dma_start(id_sb[:, :], id_d[:, :])
            emit_xt_dma(1)
            # ones columns of V (bf16 1.0) and the fp32r ones row
            for _jg in range(BT // 128):
                for _c in (_jg * VW + 64, _jg * VW + 129):
                    nc.gpsimd.memset(V[:, _c:_c + 1].bitcast(mybir.dt.uint16),
                                     0x3F80)

            # ---------------- weave queues ---------------------------------
            hard = deque()   # epilogue + Q-proj units: by current block end
            mid = deque()    # K/V-proj units: by slot njt-4 of their block
            soft = deque()   # out-proj units: deadline = end of kernel
            slots_left_global = [TOTAL_SLOTS]
            gslot = [0]
            mid_deadline = [0]

            def weave(slots_left_block, slot_idx, last_block=False):
                if hard:
                    k = -(-len(hard) // max(1, slots_left_block - 2))
                    for _ in range(min(k, len(hard))):
                        hard.popleft()()
                if mid:
                    k = -(-len(mid) // max(1, mid_deadline[0] - gslot[0] - 2))
                    for _ in range(min(k, len(mid))):
                        mid.popleft()()
                if soft and slot_idx >= 2 and (slots_left_block > 2
                                               or last_block):
                    k = -(-len(soft) // max(1, slots_left_global[0]))
                    for _ in range(min(k, len(soft))):
                        soft.popleft()()
                slots_left_global[0] -= 1
                gslot[0] += 1

            # ---------------- Q/K/V projection for one 512-token block ----
            def qkv_units(tb):
                tcols = slice(tb * TB, (tb + 1) * TB)
                ctx = {}
                units = []

                kv_start = [0]
                for which, w_sb, b_sb in (
                    ("q", wq_sb, bq_sb), ("k", wk_sb, bk_sb), ("v", wv_sb, bv_sb)
                ):
                    if which == "k":
                        kv_start[0] = len(units)
                    def mk_mm(which, w_sb, lo, hi):
                        def u():
                            if lo == 0:
                                ctx[which] = psum.tile(
                                    [128, TB], F32, tag="mm", bufs=2,
                                    name=f"ps_{which}_{tb}")
                            ps = ctx[which]
                            xt = xts[tb]
                            for ct in range(lo, hi):
                                nc.tensor.matmul(
                                    ps[:, :],
                                    w_sb[:, ct * 128:(ct + 1) * 128],
                                    xt[:, ct * TB:(ct + 1) * TB],
                                    start=(ct == 0), stop=(ct == NKT - 1),
                                )
                        return u
                    units.append(mk_mm(which, w_sb, 0, 4))
                    units.append(mk_mm(which, w_sb, 4, 8))

                    def mk_fin(which, b_sb):
                        def u():
                            ps = ctx[which]
                            if which == "q":
                                nc.vector.tensor_scalar_add(
                                    QT[:, tcols], ps[:, :], b_sb[:, :])
                            elif which == "k":
                                nc.vector.tensor_scalar_add(
                                    KT[:, tcols], ps[:, :], b_sb[:, :])
                            else:
                                vt = work.tile([128, TB], BF16, tag="vt",
                                               bufs=2, name=f"vt_{tb}")
                                ctx["vt"] = vt
                                nc.vector.tensor_scalar_add(
                                    vt[:, :], ps[:, :], b_sb[:, :])
                        return u
                    units.append(mk_fin(which, b_sb))

                def mk_tr(q4):
                    def u():
                        jg = tb * 4 + q4
                        off = jg * VW
                        tp = psum.tile([128, 128], BF16, tag="mm", bufs=2,
                                       name=f"tp_{jg}")
                        nc.tensor.transpose(
                            tp[:, :], ctx["vt"][:, q4 * 128:(q4 + 1) * 128],
                            id_sb[:, :])
                        nc.vector.tensor_copy(V[:, off:off + 64], tp[:, 0:64])
                        nc.vector.tensor_copy(V[:, off + 65:off + 129],
                                              tp[:, 64:128])
                    return u
                for q4 in range(4):
                    units.append(mk_tr(q4))
                return units[:kv_start[0]], units[kv_start[0]:]

            # ---------------- output projection for one i-block -----------
            def oproj_units(b, ib):
                i0 = b * T + ib * TB
                icols = slice(i0, i0 + TB)
                units = []
                for co in range(8):
                    def u(co=co):
                        yp = psum.tile([128, TB], F32, tag="mm", bufs=2,
                                       name=f"yp_{b}_{ib}_{co}")
                        nc.tensor.matmul(
                            yp[:, :],
                            wo_sb[:, co * 128:(co + 1) * 128],
                            ylocT[:, icols],
                            start=True, stop=True,
                        )
                        yo = work.tile([128, TB], BF16, tag="yo", bufs=3,
                                       name=f"yo_{b}_{ib}_{co}")
                        nc.vector.tensor_copy(yo[:, :], yp[:, :])
                        nc.sync.dma_start(yT_d[co * 128:(co + 1) * 128, icols],
                                          yo[:, :])
                    units.append(u)
                return units

            # ---------------- attention for one (batch, i-block) ----------
            def emit_pv(jt, pt, moff, njt, b, ots):
                jg = b * NJT + jt
                for h in range(HL):
                    off = jg * VW + h * 65
                    nc.tensor.matmul(
                        ots[h][:, moff:TB],
                        V[:, off:off + 65],
                        pt[:, h * TB + moff:(h + 1) * TB],
                        start=(jt == 0), stop=(jt == njt - 1),
                        skip_group_check=True,
                    )

            def att_block(b, ib):
                i0 = b * T + ib * TB
                icols = slice(i0, i0 + TB)
                njt = 4 * (ib + 1)
                ots = [
                    psum.tile([65, TB], F32, tag="ot", bufs=2,
                              name=f"ot_{b}_{ib}_{h}")
                    for h in range(HL)
                ]
                prev = None
                for jt in range(njt):
                    if jt == njt - 4:
                        while mid:
                            mid.popleft()()
                    jg = b * NJT + jt
                    q = jt - (njt - 4)
                    moff = 128 * q if q > 0 else 0
                    st = psum.tile([128, 2 * TB], F32, tag="st", bufs=2,
                                   name=f"st_{b}_{ib}_{jt}")
                    for h in range(HL):
                        hs = slice(h * D, (h + 1) * D)
                        nc.tensor.matmul(
                            st[:, h * TB + moff:(h + 1) * TB],
                            KT[hs, jg * 128:(jg + 1) * 128],
                            QT[hs, i0 + moff:i0 + TB],
                            start=True, stop=True,
                        )
                    pt = work.tile([128, 2 * TB], BF16, tag="pt", bufs=6,
                                   name=f"pt_{b}_{ib}_{jt}")
                    # full-tile exp even when trimmed: stale st columns get
                    # exp'd into pt but PV only reads the valid region
                    nc.scalar.activation(pt[:, :], st[:, :], AF.Exp,
                                         scale=0.125)
                    if q >= 0:
                        for h in range(HL):
                            nc.vector.tensor_mul(
                                pt[:, h * TB + moff:h * TB + moff + 128],
                                pt[:, h * TB + moff:h * TB + moff + 128],
                                tri_sb[:, :])
                    if prev is not None:
                        emit_pv(*prev, b, ots)
                    prev = (jt, pt, moff, njt)
                    weave(njt - 1 - jt, jt, b == B - 1 and ib == IB - 1)
                emit_pv(*prev, b, ots)

                # epilogue: 1/s broadcast + scale; mult reads the PSUM
                # accumulator directly (one PSUM operand is allowed).
                # Returned as units so the caller can defer emission past
                # the next block's first S matmuls.
                bcs = {}

                def rb(h):
                    s_sb = work.tile([1, TB], F32, tag="s",
                                     name=f"s_{b}_{ib}_{h}")
                    nc.vector.tensor_copy(s_sb[:, :], ots[h][64:65, :])
                    r_sb = work.tile([1, TB], F32, tag="r",
                                     name=f"r_{b}_{ib}_{h}")
                    nc.vector.reciprocal_approx_fast(r_sb[:, :], s_sb[:, :])
                    bc = work.tile([64, TB], F32, tag="bcsb",
                                   name=f"bc_{b}_{ib}_{h}")
                    nc.gpsimd.partition_broadcast(bc[:, :], r_sb[0:1, :])
                    bcs[h] = bc

                def u1():
                    # phase 1: h1's 1/s chain only -- no mult, so the DVE
                    # never head-of-line blocks on the gpsimd broadcast
                    rb(1)

                def u2():
                    rb(0)
                    yn1 = work.tile([64, TB], BF16, tag="yn1",
                                    name=f"yn_{b}_{ib}")
                    nc.vector.tensor_tensor(
                        yn1[:, :], ots[1][0:64, :], bcs[1][:, :], ALU.mult)
                    nc.sync.dma_start(ylocT[64:128, icols], yn1[:, :])
                    nc.vector.tensor_tensor(
                        ylocT[0:64, icols], ots[0][0:64, :], bcs[0][:, :],
                        ALU.mult)

                return [u1, u2]

            # ---------------- main emission schedule ----------------------
            q0, kv0 = qkv_units(0)
            for u in q0 + kv0:
                u()
            epi_units = []
            NJTS = [4 * (i % IB + 1) for i in range(NTB)]
            for b in range(B):
                for ib in range(IB):
                    blk = b * IB + ib
                    if blk + 2 < NTB:
                        emit_xt_dma(blk + 2)
                    for u in epi_units:
                        hard.append(u)
                    if blk + 1 < NTB:
                        qu, kvu = qkv_units(blk + 1)
                        hard.extend(qu)
                        mid.extend(kvu)
                        mid_deadline[0] = (gslot[0] + NJTS[blk]
                                           + NJTS[blk + 1] - 4)
                    epi_units = att_block(b, ib)
                    soft.extend(oproj_units(b, ib))
            for u in epi_units:
                u()
            while hard:
                hard.popleft()()
            while mid:
                mid.popleft()()
            while soft:
                soft.popleft()()
    nc.compile()
    return nc


def _host_inputs(x, Wq, bq, Wk, bk, Wv, bv, Wo):
    """Build the 8 per-core input maps (host-side layout prep + sharding)."""
    import ml_dtypes
    bf16 = ml_dtypes.bfloat16
    xT = np.ascontiguousarray(x.reshape(BT, C).T.astype(bf16))
    jj = np.arange(128, dtype=np.int32)[:, None]
    ii = np.arange(128, dtype=np.int32)[None, :]
    tri = (ii >= jj).astype(np.float32).astype(bf16)
    ident = np.eye(128, dtype=bf16)

    def wtile(W, rows):
        # device layout: w_sb[p, k*128 + j] = W[rows][j, k*128 + p]
        wT = W[rows, :].T.astype(bf16)                # [C, CL]
        return np.ascontiguousarray(
            wT.reshape(NKT, 128, CL).transpose(1, 0, 2).reshape(128, NKT * CL))

    in_maps = []
    for core in range(NCORES):
        rows = slice(core * CL, (core + 1) * CL)
        in_maps.append({
            "xT": xT,
            "wqT": wtile(Wq, rows),
            "wkT": wtile(Wk, rows),
            "wvT": wtile(Wv, rows),
            "woT": np.ascontiguousarray(Wo[:, rows].T.astype(bf16)),
            "bq": np.ascontiguousarray(bq[rows].reshape(CL, 1).astype(np.float32)),
            "bk": np.ascontiguousarray(bk[rows].reshape(CL, 1).astype(np.float32)),
            "bv": np.ascontiguousarray(bv[rows].reshape(CL, 1).astype(np.float32)),
            "tri": tri,
            "ident": ident,
        })
    return in_maps


_NC_CACHE = None


def _get_nc():
    global _NC_CACHE
    if _NC_CACHE is None:
        _NC_CACHE = build_nc()
    return _NC_CACHE


def _run(inputs, trace=False):
    x = np.asarray(inputs["x"], np.float32)
    in_maps = _host_inputs(
        x,
        np.asarray(inputs["Wq"], np.float32), np.asarray(inputs["bq"], np.float32),
        np.asarray(inputs["Wk"], np.float32), np.asarray(inputs["bk"], np.float32),
        np.asarray(inputs["Wv"], np.float32), np.asarray(inputs["bv"], np.float32),
        np.asarray(inputs["Wo"], np.float32),
    )
    res = run_bass_kernel_spmd(_get_nc(), in_maps, list(range(NCORES)), trace=trace)
    yT = np.zeros((C, BT), np.float64)
    for core in range(NCORES):
        yT += res.results[core]["yT"].astype(np.float64)
    y = yT.T.astype(np.float32) + np.asarray(inputs["bo"], np.float32)
    return y.reshape(B, T, C), res


def kernel(**inputs) -> np.ndarray:
    out, _ = _run(inputs, trace=False)
    return out


def _install_profile_hook():
    """Register the axon NTFF profile hook (the agent image ships the ctypes
    shim in trn_agent_boot but lacks the antenv.axon_hooks module)."""
    import types

    if "antenv.axon_hooks" in sys.modules:
        return
    sys.path.insert(0, "/root/.axon_site")
    from trn_agent_boot.trn_boot import _ntff_profile_via_ctypes

    mod = types.ModuleType("antenv.axon_hooks")
    hook = _ntff_profile_via_ctypes("/opt/axon/libaxon_pjrt.so")
    mod.get_axon_ntff_profile_hook = lambda: hook
    mod.set_axon_ntff_profile_hook = lambda h: None
    sys.modules["antenv.axon_hooks"] = mod
    import antenv

    antenv.axon_hooks = mod
    from concourse import bass_utils as _bu

    _bu.upload_artifacts = lambda tmpdir: tmpdir  # keep artifacts local


def kernel_profiled(**inputs):
    """Returns (output, exec_time_ns) using the NTFF profile of core 0."""
    _install_profile_hook()
    out, res = _run(inputs, trace=True)
    return out, res.exec_time_ns
